# revision 9
# baseline (speedup 1.0000x reference)
"""Grouped SwiGLU FFN (8 experts) — expert-parallel Bass kernel for 8 trn2 cores.

Per core (one expert): out = (silu(x@w1) * (x@w3T)) @ w2T.
  x: [T=1024, D=2048], w1: [D, H=4096], w3: [H, D], w2: [D, H].

All matmul operands are bf16 (PE runs bf16 at the same 1 col/cycle rate as
fp32r, but weight DMA halves and FWL doubles LDWEIGHTS rate); PSUM
accumulation is fp32, epilogues in fp32. End-to-end rel err ~4e-3.

Device-side (layouts pre-packed on host, zero on-device transposes):
  phase1: g^T[h, t]  = silu(w1-tile.T @ x^T) * (w3-tile.T @ x^T)  per h-tile,
          full H in one pass (g is [128, 32, 1024] bf16 = 64KB/partition)
  phase2: out^T[d,t] = sum over all 32 h-tiles of w2-tile.T @ g^T, one psum
          accumulation per (dtt, t-half); result copied + DMAd per dtt.
Matmuls are t-half-paired so each weight tile is stationary for 2 MMs.
The first 4 dt-tiles of ht=0 are split into N=128 quarter-MMs to densify
early PE activity (HAM un-throttles ~3.4us sooner).
"""

import sys

sys.path.insert(0, "/opt/trn_rl_repo")

import numpy as np
import ml_dtypes

import concourse.bass as bass
from concourse import bacc
import concourse.mybir as mybir
import concourse.tile as tile
from concourse.bass_utils import run_bass_kernel_spmd

E, T, D, H = 8, 1024, 2048, 4096
P = 128
NT = 512            # matmul moving free dim per psum bank (fp32 psum)
DT = D // P         # 16 contraction tiles over D
HT = H // P         # 32 h-tiles
TH = T // NT        # 2 t-halves
DTT = D // P        # 16 out^T row tiles
F32 = mybir.dt.float32
BF16 = mybir.dt.bfloat16

_CACHE: dict = {}


def _build_nc():
    nc = bacc.Bacc("TRN2", target_bir_lowering=False, debug=False)
    xp = nc.dram_tensor("xp", [DT, P, T], BF16, kind="ExternalInput")
    w1p = nc.dram_tensor("w1p", [HT, P, DT, P], BF16, kind="ExternalInput")
    w3p = nc.dram_tensor("w3p", [HT, P, DT, P], BF16, kind="ExternalInput")
    w2p = nc.dram_tensor("w2p", [DTT, P, HT, P], BF16, kind="ExternalInput")
    outT = nc.dram_tensor("outT", [D, T], F32, kind="ExternalOutput")

    with tile.TileContext(nc) as tc:
        with (
            tc.tile_pool(name="xpool", bufs=1) as xpool,
            tc.tile_pool(name="gpool", bufs=1) as gpool,
            tc.tile_pool(name="wpool", bufs=2) as wpool,
            tc.tile_pool(name="w2pool", bufs=2) as w2pool,
            tc.tile_pool(name="spool", bufs=2) as spool,
            tc.tile_pool(name="stpool", bufs=2) as stpool,
            tc.tile_pool(name="pspool", bufs=8, space="PSUM") as pspool,
        ):
            def load_w(ht, chunks=1):
                w1sb = wpool.tile([P, DT, P], BF16, tag="w1", name=f"w1sb_{ht}")
                w3sb = wpool.tile([P, DT, P], BF16, tag="w3", name=f"w3sb_{ht}")
                step = DT // chunks
                for c in range(chunks):
                    sl = slice(c * step, (c + 1) * step)
                    nc.sync.dma_start(w1sb[:, sl], w1p[ht, :, sl])
                    nc.sync.dma_start(w3sb[:, sl], w3p[ht, :, sl])
                return w1sb, w3sb

            # DMA issue costs ~620ns/instr on an engine queue: put x on the
            # (otherwise idle) gpsimd engine so x and weight issue streams
            # run in parallel from the first post-preamble cycle
            xsb = xpool.tile([P, DT, T], BF16, tag="x")
            for dt_i in range(DT):
                nc.gpsimd.dma_start(xsb[:, dt_i], xp[dt_i])
            w_pre = load_w(0, chunks=4)
            w_nxt = load_w(1)

            g = gpool.tile([P, HT, T], BF16, tag="g")

            def mm_quarters(ps, wsb, dt_i, th, start):
                # N=128 quarter MMs: denser early PE stream (same psum bank,
                # only the group's very first MM carries start=True)
                for q in range(4):
                    qs = slice(th * NT + q * P, th * NT + (q + 1) * P)
                    ps_q = slice(q * P, (q + 1) * P)
                    nc.tensor.matmul(
                        ps[:, ps_q],
                        lhsT=wsb[:, dt_i],
                        rhs=xsb[:, dt_i, qs],
                        start=(start and q == 0),
                        stop=False,
                        skip_group_check=True,
                    )

            for ht in range(HT):
                if ht == 0:
                    w1sb, w3sb = w_pre
                elif ht == 1:
                    w1sb, w3sb = w_nxt
                else:
                    w1sb, w3sb = w_cur
                if ht + 1 < HT:
                    if ht == 0:
                        w_cur = w_nxt
                    else:
                        w_cur = load_w(ht + 1)
                ps1 = [
                    pspool.tile([P, NT], F32, tag="ps", bufs=8, name=f"ps1_{th}")
                    for th in range(TH)
                ]
                ps3 = [
                    pspool.tile([P, NT], F32, tag="ps", bufs=8, name=f"ps3_{th}")
                    for th in range(TH)
                ]
                # ht=0: all 4 groups advance per dt chunk, so each x chunk is
                # consumed once as it arrives (~290 GB/s demand, matches DMA).
                # ht>0: all w1 groups then all w3 — frees ps1 banks (silu-only
                # readers) early so the next ht never waits on psum rotation.
                if ht == 0:
                    for dt_i in range(DT):
                        split = dt_i < 4
                        for ps_pair, wsb in ((ps1, w1sb), (ps3, w3sb)):
                            for th in range(TH):
                                if split:
                                    mm_quarters(
                                        ps_pair[th], wsb, dt_i, th,
                                        start=(dt_i == 0),
                                    )
                                else:
                                    ts = slice(th * NT, (th + 1) * NT)
                                    nc.tensor.matmul(
                                        ps_pair[th],
                                        lhsT=wsb[:, dt_i],
                                        rhs=xsb[:, dt_i, ts],
                                        start=(dt_i == 0),
                                        stop=(dt_i == DT - 1),
                                        skip_group_check=True,
                                    )
                else:
                    for ps_pair, wsb in ((ps1, w1sb), (ps3, w3sb)):
                        for dt_i in range(DT):
                            for th in range(TH):
                                ts = slice(th * NT, (th + 1) * NT)
                                nc.tensor.matmul(
                                    ps_pair[th],
                                    lhsT=wsb[:, dt_i],
                                    rhs=xsb[:, dt_i, ts],
                                    start=(dt_i == 0),
                                    stop=(dt_i == DT - 1),
                                )
                for th in range(TH):
                    ts = slice(th * NT, (th + 1) * NT)
                    sil = spool.tile([P, NT], F32, tag="sil")
                    nc.scalar.activation(
                        sil, ps1[th], mybir.ActivationFunctionType.Silu
                    )
                    nc.vector.tensor_mul(out=g[:, ht, ts], in0=sil, in1=ps3[th])

            # phase 2: out^T[dtt] = sum_ht w2tile.T @ g, full-K psum groups
            for dtt in range(DTT):
                w2sb = w2pool.tile([P, HT, P], BF16, tag="w2")
                nc.sync.dma_start(w2sb, w2p[dtt])
                po = [
                    pspool.tile([P, NT], F32, tag="ps", bufs=8, name=f"po_{th}")
                    for th in range(TH)
                ]
                for ht in range(HT):
                    for th in range(TH):
                        ts = slice(th * NT, (th + 1) * NT)
                        nc.tensor.matmul(
                            po[th],
                            lhsT=w2sb[:, ht],
                            rhs=g[:, ht, ts],
                            start=(ht == 0),
                            stop=(ht == HT - 1),
                        )
                stage = stpool.tile([P, T], F32, tag="stage")
                # drain the two t-halves on both engines in parallel
                nc.scalar.copy(stage[:, 0:NT], po[0])
                nc.vector.tensor_copy(out=stage[:, NT:T], in_=po[1])
                dsl = slice(dtt * P, (dtt + 1) * P)
                if dtt < DTT - 1:
                    nc.sync.dma_start(outT[dsl], stage)
                else:
                    # split the last DMA so the tail drains in halves
                    nc.sync.dma_start(outT[dsl, 0:NT], stage[:, 0:NT])
                    nc.sync.dma_start(outT[dsl, NT:T], stage[:, NT:T])
    nc.compile()
    return nc


def _pack_inputs(x, w1, w2, w3):
    """Per-expert host-side packing into DMA-linear bf16 layouts."""
    bf = ml_dtypes.bfloat16
    in_maps = []
    for e in range(E):
        xe = np.asarray(x[e], dtype=np.float32).astype(bf)
        w1e = np.asarray(w1[e], dtype=np.float32).astype(bf)
        w2e = np.asarray(w2[e], dtype=np.float32).astype(bf)
        w3e = np.asarray(w3[e], dtype=np.float32).astype(bf)
        # xp[dt, p, t] = x[t, dt*128+p]
        xpk = np.ascontiguousarray(xe.reshape(T, DT, P).transpose(1, 2, 0))
        # w1p[ht, p, dt, h] = w1[dt*128+p, ht*128+h]
        w1pk = np.ascontiguousarray(
            w1e.reshape(DT, P, HT, P).transpose(2, 1, 0, 3)
        )
        # w3p[ht, p, dt, h] = w3[ht*128+h, dt*128+p]
        w3pk = np.ascontiguousarray(
            w3e.reshape(HT, P, DT, P).transpose(0, 3, 2, 1)
        )
        # w2p[dtt, p, ht, d] = w2[dtt*128+d, ht*128+p]  (partition-first tile)
        w2pk = np.ascontiguousarray(
            w2e.reshape(DTT, P, HT, P).transpose(0, 3, 2, 1)
        )
        in_maps.append({"xp": xpk, "w1p": w1pk, "w3p": w3pk, "w2p": w2pk})
    return in_maps


def kernel(x, w1, w2, w3, _trace=False, _trace_kwargs=None):
    if "nc" not in _CACHE:
        _CACHE["nc"] = _build_nc()
    nc = _CACHE["nc"]
    in_maps = _pack_inputs(x, w1, w2, w3)
    kw = {}
    if _trace:
        kw = {"trace": True}
        if _trace_kwargs:
            kw.update(_trace_kwargs)
    res = run_bass_kernel_spmd(nc, in_maps, core_ids=list(range(E)), **kw)
    out = np.empty((E, T, D), dtype=np.float32)
    for e in range(E):
        out[e] = res.results[e]["outT"].T
    if _trace:
        _CACHE["last_results"] = res
    return out


# revision 17
# speedup vs baseline: 1.0122x; 1.0122x over previous
"""Grouped SwiGLU FFN (8 experts) — expert-parallel Bass kernel for 8 trn2 cores.

Per core (one expert): out = (silu(x@w1) * (x@w3T)) @ w2T.
  x: [T=1024, D=2048], w1: [D, H=4096], w3: [H, D], w2: [D, H].

All matmul operands are bf16 (PE runs bf16 at the same 1 col/cycle rate as
fp32r, but weight DMA halves and FWL doubles LDWEIGHTS rate); PSUM
accumulation is fp32, epilogues in fp32. End-to-end rel err ~4e-3.

Device-side (layouts pre-packed on host, zero on-device transposes):
  phase1: g^T[h, t]  = silu(w1-tile.T @ x^T) * (w3-tile.T @ x^T)  per h-tile,
          full H in one pass (g is [128, 32, 1024] bf16 = 64KB/partition)
  phase2: out^T[d,t] = sum over all 32 h-tiles of w2-tile.T @ g^T, one psum
          accumulation per (dtt, t-half); result copied + DMAd per dtt.
Matmuls are t-half-paired so each weight tile is stationary for 2 MMs.
The first 4 dt-tiles of ht=0 are split into N=128 quarter-MMs to densify
early PE activity (HAM un-throttles ~3.4us sooner).
"""

import sys

sys.path.insert(0, "/opt/trn_rl_repo")

import numpy as np
import ml_dtypes

import concourse.bass as bass
from concourse import bacc
import concourse.mybir as mybir
import concourse.tile as tile
from concourse.bass_utils import run_bass_kernel_spmd

E, T, D, H = 8, 1024, 2048, 4096
P = 128
NT = 512            # matmul moving free dim per psum bank (fp32 psum)
DT = D // P         # 16 contraction tiles over D
HT = H // P         # 32 h-tiles
TH = T // NT        # 2 t-halves
DTT = D // P        # 16 out^T row tiles
F32 = mybir.dt.float32
BF16 = mybir.dt.bfloat16

_CACHE: dict = {}


def _build_nc():
    nc = bacc.Bacc("TRN2", target_bir_lowering=False, debug=False)
    xp = nc.dram_tensor("xp", [P, DT, T], BF16, kind="ExternalInput")
    # w1 and w3 packed together: one DMA issue (~620ns of sync-engine time
    # each) covers both weight tiles of an ht
    wp = nc.dram_tensor("wp", [HT, P, 2, DT, P], BF16, kind="ExternalInput")
    w2p = nc.dram_tensor("w2p", [DTT, P, HT, P], BF16, kind="ExternalInput")
    outT = nc.dram_tensor("outT", [D, T], F32, kind="ExternalOutput")

    with tile.TileContext(nc) as tc:
        with (
            tc.tile_pool(name="xpool", bufs=1) as xpool,
            tc.tile_pool(name="gpool", bufs=1) as gpool,
            tc.tile_pool(name="wpool", bufs=2) as wpool,
            tc.tile_pool(name="w2pool", bufs=2) as w2pool,
            tc.tile_pool(name="spool", bufs=2) as spool,
            tc.tile_pool(name="stpool", bufs=2) as stpool,
            tc.tile_pool(name="pspool", bufs=8, space="PSUM") as pspool,
        ):
            def load_w(ht, chunk=None):
                if chunk is None:
                    wsb = wpool.tile([P, 2, DT, P], BF16, tag="w", name=f"wsb_{ht}")
                    nc.sync.dma_start(wsb, wp[ht])
                    return wsb
                wsb, sl = chunk
                nc.sync.dma_start(wsb[:, :, sl], wp[ht, :, :, sl])
                return wsb

            # startup: single sync queue issues one DMA per ~620ns, so
            # interleave the ht=0 weight chunks with x chunks in exactly the
            # order the ht=0 matmuls consume them
            xsb = xpool.tile([P, DT, T], BF16, tag="x")
            w_pre = wpool.tile([P, 2, DT, P], BF16, tag="w", name="wsb_0")
            xq = 0

            def issue_x(n):
                nonlocal xq
                sl = slice(xq, xq + n)
                nc.sync.dma_start(xsb[:, sl], xp[:, sl])
                xq += n

            for c in range(4):
                load_w(0, chunk=(w_pre, slice(c * 4, (c + 1) * 4)))
                issue_x(1)
                issue_x(1)
            w_nxt = load_w(1)
            while xq < DT:
                issue_x(2)

            g = gpool.tile([P, HT, T], BF16, tag="g")

            def mm_quarters(ps, wsb, dt_i, th, start):
                # N=128 quarter MMs: denser early PE stream (same psum bank,
                # only the group's very first MM carries start=True)
                for q in range(4):
                    qs = slice(th * NT + q * P, th * NT + (q + 1) * P)
                    ps_q = slice(q * P, (q + 1) * P)
                    nc.tensor.matmul(
                        ps[:, ps_q],
                        lhsT=wsb[:, dt_i],
                        rhs=xsb[:, dt_i, qs],
                        start=(start and q == 0),
                        stop=False,
                        skip_group_check=True,
                    )

            for ht in range(HT):
                if ht == 0:
                    wsb = w_pre
                elif ht == 1:
                    wsb = w_nxt
                else:
                    wsb = w_cur
                if 1 <= ht < HT - 1:
                    w_cur = load_w(ht + 1)
                w1sb, w3sb = wsb[:, 0], wsb[:, 1]
                ps1 = [
                    pspool.tile([P, NT], F32, tag="ps", bufs=8, name=f"ps1_{th}")
                    for th in range(TH)
                ]
                ps3 = [
                    pspool.tile([P, NT], F32, tag="ps", bufs=8, name=f"ps3_{th}")
                    for th in range(TH)
                ]
                # ht=0: all 4 groups advance per dt chunk, so each x chunk is
                # consumed once as it arrives (~290 GB/s demand, matches DMA).
                # ht>0: all w1 groups then all w3 — frees ps1 banks (silu-only
                # readers) early so the next ht never waits on psum rotation.
                if ht == 0:
                    for dt_i in range(DT):
                        split = dt_i < 4
                        for ps_pair, wsb in ((ps1, w1sb), (ps3, w3sb)):
                            for th in range(TH):
                                if split:
                                    mm_quarters(
                                        ps_pair[th], wsb, dt_i, th,
                                        start=(dt_i == 0),
                                    )
                                else:
                                    ts = slice(th * NT, (th + 1) * NT)
                                    nc.tensor.matmul(
                                        ps_pair[th],
                                        lhsT=wsb[:, dt_i],
                                        rhs=xsb[:, dt_i, ts],
                                        start=(dt_i == 0),
                                        stop=(dt_i == DT - 1),
                                        skip_group_check=True,
                                    )
                else:
                    for ps_pair, wsb in ((ps1, w1sb), (ps3, w3sb)):
                        for dt_i in range(DT):
                            for th in range(TH):
                                ts = slice(th * NT, (th + 1) * NT)
                                nc.tensor.matmul(
                                    ps_pair[th],
                                    lhsT=wsb[:, dt_i],
                                    rhs=xsb[:, dt_i, ts],
                                    start=(dt_i == 0),
                                    stop=(dt_i == DT - 1),
                                )
                for th in range(TH):
                    ts = slice(th * NT, (th + 1) * NT)
                    sil = spool.tile([P, NT], F32, tag="sil")
                    nc.scalar.activation(
                        sil, ps1[th], mybir.ActivationFunctionType.Silu
                    )
                    nc.vector.tensor_mul(out=g[:, ht, ts], in0=sil, in1=ps3[th])

            # phase 2: out^T[dtt] = sum_ht w2tile.T @ g, full-K psum groups
            for dtt in range(DTT):
                w2sb = w2pool.tile([P, HT, P], BF16, tag="w2")
                nc.sync.dma_start(w2sb, w2p[dtt])
                po = [
                    pspool.tile([P, NT], F32, tag="ps", bufs=8, name=f"po_{th}")
                    for th in range(TH)
                ]
                for ht in range(HT):
                    for th in range(TH):
                        ts = slice(th * NT, (th + 1) * NT)
                        nc.tensor.matmul(
                            po[th],
                            lhsT=w2sb[:, ht],
                            rhs=g[:, ht, ts],
                            start=(ht == 0),
                            stop=(ht == HT - 1),
                        )
                stage = stpool.tile([P, T], F32, tag="stage")
                # drain the two t-halves on both engines in parallel
                nc.scalar.copy(stage[:, 0:NT], po[0])
                nc.vector.tensor_copy(out=stage[:, NT:T], in_=po[1])
                dsl = slice(dtt * P, (dtt + 1) * P)
                if dtt < DTT - 1:
                    nc.sync.dma_start(outT[dsl], stage)
                else:
                    # split the last DMA so the tail drains in halves
                    nc.sync.dma_start(outT[dsl, 0:NT], stage[:, 0:NT])
                    nc.sync.dma_start(outT[dsl, NT:T], stage[:, NT:T])
    nc.compile()
    return nc


def _pack_inputs(x, w1, w2, w3):
    """Per-expert host-side packing into DMA-linear bf16 layouts."""
    bf = ml_dtypes.bfloat16
    in_maps = []
    for e in range(E):
        xe = np.asarray(x[e], dtype=np.float32).astype(bf)
        w1e = np.asarray(w1[e], dtype=np.float32).astype(bf)
        w2e = np.asarray(w2[e], dtype=np.float32).astype(bf)
        w3e = np.asarray(w3[e], dtype=np.float32).astype(bf)
        # xp[p, dt, t] = x[t, dt*128+p]  (matches the xsb SBUF layout)
        xpk = np.ascontiguousarray(xe.reshape(T, DT, P).transpose(2, 1, 0))
        # wp[ht, p, 0, dt, h] = w1[dt*128+p, ht*128+h]
        # wp[ht, p, 1, dt, h] = w3[ht*128+h, dt*128+p]
        w1pk = w1e.reshape(DT, P, HT, P).transpose(2, 1, 0, 3)
        w3pk = w3e.reshape(HT, P, DT, P).transpose(0, 3, 2, 1)
        wpk = np.ascontiguousarray(np.stack([w1pk, w3pk], axis=2))
        # w2p[dtt, p, ht, d] = w2[dtt*128+d, ht*128+p]  (partition-first tile)
        w2pk = np.ascontiguousarray(
            w2e.reshape(DTT, P, HT, P).transpose(0, 3, 2, 1)
        )
        in_maps.append({"xp": xpk, "wp": wpk, "w2p": w2pk})
    return in_maps


def kernel(x, w1, w2, w3, _trace=False, _trace_kwargs=None):
    if "nc" not in _CACHE:
        _CACHE["nc"] = _build_nc()
    nc = _CACHE["nc"]
    in_maps = _pack_inputs(x, w1, w2, w3)
    kw = {}
    if _trace:
        kw = {"trace": True}
        if _trace_kwargs:
            kw.update(_trace_kwargs)
    res = run_bass_kernel_spmd(nc, in_maps, core_ids=list(range(E)), **kw)
    out = np.empty((E, T, D), dtype=np.float32)
    for e in range(E):
        out[e] = res.results[e]["outT"].T
    if _trace:
        _CACHE["last_results"] = res
    return out
